# revision 17
# baseline (speedup 1.0000x reference)
"""MoE routing kernel for Trainium2 (8 NeuronCores, expert-parallel).

Problem: top-2-of-8 expert MLP with squared-ReLU, d_model=1024, d_ff=1024,
N=8192 tokens. Strategy: the router (softmax + top-2, ~0.2% of FLOPs) runs
on host in float64; tokens are dispatched on host (gather + sqrt(combine-
weight) scaling — relu(sqrt(w)*z)^2 == w*relu(z)^2, so the combine weight
folds into the input and the device kernel is a plain 2-layer MLP). One
expert per core; capacity = max expert count rounded to 8 (the SPMD floor —
every static uniform capacity must cover the largest expert).

The device pipeline runs in float16: the PE matmul rate is identical to
f32r (1 col/cycle), but fp16 weights get Fast-Weight-Load (LDWEIGHTS
~53 ns vs 107 ns, fully hidden under >=128-col matmuls) and every DMA
stream halves (w1+w2 4 MiB, x 4.2 MiB, y 4.2 MiB per core), which removes
the weight-stream stalls and most of the head/tail latency. fp16 quant
error is ~1e-4 overall — far inside the 2e-2 gate. Host scatter-adds the
per-core outputs in f32.
"""

import sys

if "/opt/trn_rl_repo" not in sys.path:
    sys.path.insert(0, "/opt/trn_rl_repo")

import numpy as np

import bass_rust
import concourse.bass as bass
import concourse.tile as tile
import concourse.tile_utils as tile_utils
from concourse import mybir
from concourse.bass_utils import run_bass_kernel_spmd
from concourse.vector_clock import ScopedClock

NUM_EXPERTS = 8
TOP_K = 2
D_MODEL = 1024
D_FF = 1024
N_CORES = 8
KC = D_MODEL // 128
FT = D_FF // 128
DT = D_MODEL // 128

# Cayman has 208 KiB/partition usable; the stock constant leaves 16 KiB idle.
tile_utils.max_sbuf_usage = 208 * 1024

# ---------------------------------------------------------------------------
# Compat: this container's walrus rejects instructions carrying more than one
# sem wait ("Too many sync wait commands"). Replace the TileContext final
# drain with single-wait SP nops, and post-process the module so every
# instruction carries at most one (monotonic) wait.
# ---------------------------------------------------------------------------


def _patched_drain_and_barrier(self, tick_clock, wait_clock):
    probe = self.nc.sync.nop(nofuse=True)
    wait_clock.add_sem_waits(probe.ins, ScopedClock({None: tick_clock.global_clock}))
    si = probe.ins.sync_info
    waits = list(si.on_wait) if si is not None else []
    updates = list(si.on_update) if si is not None else []
    if len(waits) > 1:
        probe.ins.sync_info = bass_rust.SyncInfo(on_wait=[waits[0]], on_update=updates)
        for w in waits[1:]:
            extra = self.nc.sync.nop(nofuse=True)
            extra.ins.sync_info = bass_rust.SyncInfo(on_wait=[w], on_update=[])
    self.nc.sync.drain()
    self.nc.all_engine_barrier()
    assert self.sems is not None
    popped = self.nc._tile_sem_poison_stack.pop()
    assert popped is self._sem_poison
    self.nc.clear_and_free_semaphores(list(self.sems.allocated().values()))
    self.nc.all_engine_barrier()


tile.TileContext._drain_and_barrier = _patched_drain_and_barrier


def split_excess_waits(nc, limit=1):
    for fn in nc.m.functions:
        for bb in fn.blocks:
            il = bb.instructions
            i = 0
            while i < len(il):
                inst = il[i]
                si = inst.sync_info
                if si is not None and len(si.on_wait) > limit:
                    waits = list(si.on_wait)
                    movable = [w for w in waits if "ge" in (w.wait_mode or "")]
                    pinned = [w for w in waits if w not in movable]
                    keep_n = max(0, limit - len(pinned))
                    if keep_n:
                        keep = pinned + movable[len(movable) - keep_n :]
                        extra = movable[: len(movable) - keep_n]
                    else:
                        keep, extra = pinned, movable
                    if not extra:
                        i += 1
                        continue
                    nops = []
                    for w in extra:
                        nop = mybir.InstNoOp(
                            name=nc.get_next_instruction_name(), ins=[], outs=[]
                        )
                        nop.engine = inst.engine
                        nop.sync_info = bass_rust.SyncInfo(on_wait=[w], on_update=[])
                        nops.append(nop)
                    inst.sync_info = bass_rust.SyncInfo(
                        on_wait=keep, on_update=list(si.on_update)
                    )
                    for j, nop in enumerate(nops):
                        il.insert(i + j, nop)
                    i += len(nops)
                i += 1


# ---------------------------------------------------------------------------
# Capacities and token blocks.
# ---------------------------------------------------------------------------

F16 = mybir.dt.float16
F32 = mybir.dt.float32


def _token_blocks(cap):
    """Lead block 256 (prime the pipeline while weights stream), 512 steady
    state (one full PSUM bank), small tail for a fast drain."""
    assert cap % 8 == 0 and cap >= 768
    sizes = [256, 256]
    rem = cap - 512
    while rem >= 616:
        sizes.append(512)
        rem -= 512
    if rem >= 208:
        sizes.extend([rem - 104, 104])
    else:
        sizes.append(rem)
    blocks, t = [], 0
    for tb in sizes:
        blocks.append((t, tb))
        t += tb
    assert t == cap, (cap, sizes)
    return blocks


def _chunks(c0, c1, step):
    out = []
    while c0 < c1:
        out.append((c0, min(c0 + step, c1)))
        c0 = out[-1][1]
    return out


# 2048 fp16 cols = 4 KiB per partition row — the DGE's best packet size.
DMA_CHUNK = 2048


def build_program(cap):
    nc = bass.Bass("TRN2", target_bir_lowering=False, debug=False, num_devices=N_CORES)
    # xP: host-packed [128, KC*cap]; token block (t0,tb) occupies columns
    # [KC*t0, KC*(t0+tb)) laid out [p, (kc t)]. yP likewise [p, (dt t)].
    # w1/w2 host-prepacked [128, 8192]: col (m*1024 + kc*128 + j) holds
    # W[kc*128+p, m*128+j] — identical layout to the SBUF tile, so DMAs are
    # plain 2D copies in any chunking.
    xP = nc.declare_dram_parameter("xP", [128, KC * cap], F16, isOutput=False)
    w1 = nc.declare_dram_parameter("w1", [128, FT * D_MODEL], F16, isOutput=False)
    w2 = nc.declare_dram_parameter("w2", [128, DT * D_FF], F16, isOutput=False)
    yP = nc.declare_dram_parameter("yP", [128, DT * cap], F16, isOutput=True)

    blocks = _token_blocks(cap)
    nb = len(blocks)

    with tile.TileContext(nc) as tc:
        with (
            tc.tile_pool(name="wpool", bufs=1) as wpool,
            tc.tile_pool(name="xpool", bufs=nb) as xpool,
            tc.tile_pool(name="mpool", bufs=2) as mpool,
            tc.tile_pool(name="tpool", bufs=4) as tpool,
            tc.tile_pool(name="opool", bufs=nb) as opool,
            tc.tile_pool(name="psum", bufs=3, space="PSUM") as psum_pool,
        ):
            w1_sb = wpool.tile([128, FT * D_MODEL], F16, tag="w1")
            w2_sb = wpool.tile([128, DT * D_FF], F16, tag="w2")

            # Warm-up: the PE clock is gated to 0.65-1.2 GHz until the HAM
            # sees ~3.4 us of sustained activity, and the DGE doesn't move
            # the first bytes until ~12 us (NEFF preamble ~7 us + ~3 us
            # doorbell-to-flow latency + first chunk). Fill the whole wait
            # with dependency-free fp32 matmuls (4 cy/col, ~0.9-1.6 us each
            # below full clock) so the PE hits the first real matmul ramped
            # and exactly when its data lands. memsets go on the vector
            # engine so gpsimd's first queue entries are the w1 doorbells.
            warm_a = wpool.tile([128, 128], F32, tag="warm_a")
            warm_x = wpool.tile([128, 256], F32, tag="warm_x")
            nc.vector.memset(warm_a[:], 0.0)
            nc.vector.memset(warm_x[:], 0.0)
            wp = psum_pool.tile([128, 256], F32, tag="warm", bufs=1)
            for _ in range(9):
                nc.tensor.matmul(wp[:], warm_a[:], warm_x[:], start=True, stop=True)

            # Early-phase DMA schedule (three ~135 GB/s queues, flow starts
            # ~10.5 us): sync carries x0 then x1/x2/... (x0's first 1024-col
            # piece lands ~12.4 us = warm-up end, so the PE starts on time);
            # gpsimd carries w1 c0-c3 then w2 then the output blocks; ACT
            # carries w1 c4-c7 then only relus. Output tile ft of layer 1
            # needs w1 chunk ft, so the two w1 queues stay just ahead of the
            # PE's ~0.85 us/tile pace through block 0.
            xs = {}
            t0b, tb0 = blocks[0]
            x0_sb = xpool.tile([128, KC * tb0], F16, tag="x", name="x0")
            xs[0] = x0_sb
            for c0, c1 in _chunks(0, KC * tb0, 1024):
                nc.sync.dma_start(x0_sb[:, c0:c1], xP[:, c0:c1])
            # ACT's queue starts flowing ~1 us before gpsimd's, so it gets
            # chunk 0 (the first tile the PE needs) plus the odd chunks.
            for m in (0, 1, 3, 5, 7):
                nc.scalar.dma_start(
                    w1_sb[:, m * 1024 : (m + 1) * 1024], w1[:, m * 1024 : (m + 1) * 1024]
                )
            for m in (2, 4, 6):
                nc.gpsimd.dma_start(
                    w1_sb[:, m * 1024 : (m + 1) * 1024], w1[:, m * 1024 : (m + 1) * 1024]
                )
            # w2 isn't needed until the first layer-2 block (~33 us); split it
            # over the two weight queues behind w1 so it never competes with
            # the x stream on sync.
            nc.gpsimd.dma_start(w2_sb[:, 0:2048], w2[:, 0:2048])
            nc.gpsimd.dma_start(w2_sb[:, 2048:4096], w2[:, 2048:4096])
            nc.scalar.dma_start(w2_sb[:, 4096:6144], w2[:, 4096:6144])
            nc.scalar.dma_start(w2_sb[:, 6144:8192], w2[:, 6144:8192])

            # Software-pipelined emission: the PE stream is in-order, so
            # emit L1(b+1) before L2(b) — the PE always has layer-1 work
            # while layer-2 weights / x blocks are still streaming.
            mids = {}

            def l1(bi):
                t0, tb = blocks[bi]
                if bi in xs:
                    x_sb = xs.pop(bi)
                else:
                    x_sb = xpool.tile([128, KC * tb], F16, tag="x", name=f"x{bi}")
                    for c0, c1 in _chunks(0, KC * tb, DMA_CHUNK):
                        nc.sync.dma_start(
                            x_sb[:, c0:c1], xP[:, KC * t0 + c0 : KC * t0 + c1]
                        )
                mid_sb = mpool.tile([128, FT * tb], F16, tag="mid", name=f"mid{bi}")
                mids[bi] = mid_sb
                for ft in range(FT):
                    ps = psum_pool.tile([128, tb], F32, tag="ps", name=f"ps{bi}_{ft}")
                    for kc in range(KC):
                        nc.tensor.matmul(
                            ps[:],
                            w1_sb[
                                :,
                                ft * D_MODEL + kc * 128 : ft * D_MODEL + kc * 128 + 128,
                            ],
                            x_sb[:, kc * tb : (kc + 1) * tb],
                            start=(kc == 0),
                            stop=(kc == KC - 1),
                        )
                    tmp = tpool.tile([128, tb], F16, tag="tmp", name=f"tmp{bi}_{ft}")
                    nc.scalar.activation(
                        tmp[:], ps[:], mybir.ActivationFunctionType.Relu
                    )
                    nc.vector.tensor_mul(
                        mid_sb[:, ft * tb : (ft + 1) * tb], tmp[:], tmp[:]
                    )

            def l2(bi):
                t0, tb = blocks[bi]
                mid_sb = mids.pop(bi)
                o_sb = opool.tile([128, DT * tb], F16, tag="o", name=f"o{bi}")
                for dt_ in range(DT):
                    ps2 = psum_pool.tile(
                        [128, tb], F32, tag="ps2", name=f"ps2{bi}_{dt_}"
                    )
                    for fc in range(FT):
                        nc.tensor.matmul(
                            ps2[:],
                            w2_sb[
                                :, dt_ * D_FF + fc * 128 : dt_ * D_FF + fc * 128 + 128
                            ],
                            mid_sb[:, fc * tb : (fc + 1) * tb],
                            start=(fc == 0),
                            stop=(fc == FT - 1),
                        )
                    nc.vector.tensor_copy(o_sb[:, dt_ * tb : (dt_ + 1) * tb], ps2[:])
                    if bi == nb - 1 and dt_ == DT // 2 - 1:
                        # tail block: ship the first half mid-block so only
                        # half the (tiny) output DMA trails the last matmul
                        nc.sync.dma_start(
                            yP[:, DT * t0 : DT * t0 + 4 * tb],
                            o_sb[:, : 4 * tb],
                        )
                if bi == nb - 1:
                    nc.sync.dma_start(
                        yP[:, DT * t0 + 4 * tb : DT * (t0 + tb)],
                        o_sb[:, 4 * tb : DT * tb],
                    )
                else:
                    for c0, c1 in _chunks(0, DT * tb, DMA_CHUNK):
                        nc.gpsimd.dma_start(
                            yP[:, DT * t0 + c0 : DT * t0 + c1], o_sb[:, c0:c1]
                        )

            LA = 1  # mid tiles live LA+1 blocks -> mpool bufs = LA+1
            for step in range(nb + LA):
                if step < nb:
                    l1(step)
                if step >= LA:
                    l2(step - LA)

    split_excess_waits(nc, limit=1)
    return nc


_PROGRAM_CACHE = {}


def _get_program(cap):
    if cap not in _PROGRAM_CACHE:
        _PROGRAM_CACHE[cap] = build_program(cap)
    return _PROGRAM_CACHE[cap]


# ---------------------------------------------------------------------------
# Host side: routing, dispatch, combine.
# ---------------------------------------------------------------------------


def _pack_blocked(aT, cap, blocks, dtype):
    """[1024, cap] feature-major -> [128, 8*cap], each token block laid out
    [p, (g t)] so the device moves one contiguous chunk per block."""
    g = aT.shape[0] // 128
    out = np.zeros((128, g * cap), dtype)
    for t0, tb in blocks:
        out[:, g * t0 : g * (t0 + tb)] = (
            aT[:, t0 : t0 + tb]
            .reshape(g, 128, tb)
            .transpose(1, 0, 2)
            .reshape(128, g * tb)
        )
    return out


def _unpack_blocked(aP, cap, blocks):
    g = aP.shape[1] // cap
    out = np.empty((g * 128, cap), np.float32)
    for t0, tb in blocks:
        blk = aP[:, g * t0 : g * (t0 + tb)].astype(np.float32).reshape(128, g, tb)
        out[:, t0 : t0 + tb] = blk.transpose(1, 0, 2).reshape(g * 128, tb)
    return out


def _prep_weight(w):
    """[K, M] -> [128, (mt kc j)] fp16: col mt*1024 + kc*128 + j holds
    W[kc*128 + p, mt*128 + j]."""
    k, m = w.shape
    return np.ascontiguousarray(
        w.reshape(k // 128, 128, m // 128, 128)
        .transpose(1, 2, 0, 3)
        .reshape(128, k * m // 128),
        dtype=np.float16,
    )


def kernel(x, Wr, W1, W2, _trace=False):
    x = np.asarray(x)
    Wr = np.asarray(Wr)
    W1 = np.asarray(W1)
    W2 = np.asarray(W2)
    B, T, C = x.shape
    N = B * T
    xf = np.ascontiguousarray(x.reshape(N, C), dtype=np.float32)

    # Router in float64 (matches jax f32 top_k selections; verified).
    logits = xf.astype(np.float64) @ Wr.astype(np.float64)
    logits -= logits.max(axis=-1, keepdims=True)
    p = np.exp(logits)
    p /= p.sum(axis=-1, keepdims=True)
    idx = np.argsort(-p, axis=-1, kind="stable")[:, :TOP_K]  # [N, K]
    wts = np.take_along_axis(p, idx, axis=-1)  # [N, K]

    # Dispatch list sorted by expert.
    flat_e = idx.ravel()
    order = np.argsort(flat_e, kind="stable")
    tok_of_pair = np.repeat(np.arange(N), TOP_K)[order]
    w_of_pair = wts.ravel()[order]
    counts = np.bincount(flat_e, minlength=NUM_EXPERTS)
    starts = np.concatenate([[0], np.cumsum(counts)[:-1]])

    cap = int(max(512, -(-int(counts.max()) // 8) * 8))
    blocks = _token_blocks(cap)

    in_maps = []
    toks_per_e = []
    for e in range(NUM_EXPERTS):
        s, c = int(starts[e]), int(counts[e])
        toks = tok_of_pair[s : s + c]
        toks_per_e.append(toks)
        ws = w_of_pair[s : s + c].astype(np.float32)
        xg = xf[toks] * np.sqrt(ws)[:, None]
        xTe = np.zeros((C, cap), np.float32)
        xTe[:, :c] = xg.T
        in_maps.append(
            {
                "xP": _pack_blocked(xTe, cap, blocks, np.float16),
                "w1": _prep_weight(W1[e]),
                "w2": _prep_weight(W2[e]),
            }
        )

    nc = _get_program(cap)
    res = run_bass_kernel_spmd(nc, in_maps, core_ids=list(range(N_CORES)), trace=_trace)

    out = np.zeros((N, C), np.float32)
    for e in range(NUM_EXPERTS):
        c = int(counts[e])
        if c:
            yT = _unpack_blocked(res.results[e]["yP"], cap, blocks)
            out[toks_per_e[e]] += yT[:, :c].T
    if _trace:
        kernel._last_exec_time_ns = res.exec_time_ns
    return out.reshape(B, T, C)


# revision 20
# speedup vs baseline: 1.0221x; 1.0221x over previous
"""MoE routing kernel for Trainium2 (8 NeuronCores, expert-parallel).

Problem: top-2-of-8 expert MLP with squared-ReLU, d_model=1024, d_ff=1024,
N=8192 tokens. Strategy: the router (softmax + top-2, ~0.2% of FLOPs) runs
on host in float64; tokens are dispatched on host (gather + sqrt(combine-
weight) scaling — relu(sqrt(w)*z)^2 == w*relu(z)^2, so the combine weight
folds into the input and the device kernel is a plain 2-layer MLP). One
expert per core; capacity = max expert count rounded to 8 (the SPMD floor —
every static uniform capacity must cover the largest expert).

The device pipeline runs in float16: the PE matmul rate is identical to
f32r (1 col/cycle), but fp16 weights get Fast-Weight-Load (LDWEIGHTS
~53 ns vs 107 ns, fully hidden under >=128-col matmuls) and every DMA
stream halves (w1+w2 4 MiB, x 4.2 MiB, y 4.2 MiB per core), which removes
the weight-stream stalls and most of the head/tail latency. fp16 quant
error is ~1e-4 overall — far inside the 2e-2 gate. Host scatter-adds the
per-core outputs in f32.
"""

import sys

if "/opt/trn_rl_repo" not in sys.path:
    sys.path.insert(0, "/opt/trn_rl_repo")

import numpy as np

import bass_rust
import concourse.bass as bass
import concourse.tile as tile
import concourse.tile_utils as tile_utils
from concourse import mybir
from concourse.bass_utils import run_bass_kernel_spmd
from concourse.vector_clock import ScopedClock

NUM_EXPERTS = 8
TOP_K = 2
D_MODEL = 1024
D_FF = 1024
N_CORES = 8
KC = D_MODEL // 128
FT = D_FF // 128
DT = D_MODEL // 128

# Cayman has 208 KiB/partition usable; the stock constant leaves 16 KiB idle.
tile_utils.max_sbuf_usage = 208 * 1024

# ---------------------------------------------------------------------------
# Compat: this container's walrus rejects instructions carrying more than one
# sem wait ("Too many sync wait commands"). Replace the TileContext final
# drain with single-wait SP nops, and post-process the module so every
# instruction carries at most one (monotonic) wait.
# ---------------------------------------------------------------------------


def _patched_drain_and_barrier(self, tick_clock, wait_clock):
    probe = self.nc.sync.nop(nofuse=True)
    wait_clock.add_sem_waits(probe.ins, ScopedClock({None: tick_clock.global_clock}))
    si = probe.ins.sync_info
    waits = list(si.on_wait) if si is not None else []
    updates = list(si.on_update) if si is not None else []
    if len(waits) > 1:
        probe.ins.sync_info = bass_rust.SyncInfo(on_wait=[waits[0]], on_update=updates)
        for w in waits[1:]:
            extra = self.nc.sync.nop(nofuse=True)
            extra.ins.sync_info = bass_rust.SyncInfo(on_wait=[w], on_update=[])
    self.nc.sync.drain()
    self.nc.all_engine_barrier()
    assert self.sems is not None
    popped = self.nc._tile_sem_poison_stack.pop()
    assert popped is self._sem_poison
    self.nc.clear_and_free_semaphores(list(self.sems.allocated().values()))
    self.nc.all_engine_barrier()


tile.TileContext._drain_and_barrier = _patched_drain_and_barrier


def split_excess_waits(nc, limit=1):
    for fn in nc.m.functions:
        for bb in fn.blocks:
            il = bb.instructions
            i = 0
            while i < len(il):
                inst = il[i]
                si = inst.sync_info
                if si is not None and len(si.on_wait) > limit:
                    waits = list(si.on_wait)
                    movable = [w for w in waits if "ge" in (w.wait_mode or "")]
                    pinned = [w for w in waits if w not in movable]
                    keep_n = max(0, limit - len(pinned))
                    if keep_n:
                        keep = pinned + movable[len(movable) - keep_n :]
                        extra = movable[: len(movable) - keep_n]
                    else:
                        keep, extra = pinned, movable
                    if not extra:
                        i += 1
                        continue
                    nops = []
                    for w in extra:
                        nop = mybir.InstNoOp(
                            name=nc.get_next_instruction_name(), ins=[], outs=[]
                        )
                        nop.engine = inst.engine
                        nop.sync_info = bass_rust.SyncInfo(on_wait=[w], on_update=[])
                        nops.append(nop)
                    inst.sync_info = bass_rust.SyncInfo(
                        on_wait=keep, on_update=list(si.on_update)
                    )
                    for j, nop in enumerate(nops):
                        il.insert(i + j, nop)
                    i += len(nops)
                i += 1


# ---------------------------------------------------------------------------
# Capacities and token blocks.
# ---------------------------------------------------------------------------

F16 = mybir.dt.float16
F32 = mybir.dt.float32


def _token_blocks(cap):
    """Lead block 256 (prime the pipeline while weights stream), 512 steady
    state (one full PSUM bank), small tail for a fast drain."""
    assert cap % 8 == 0 and cap >= 512
    sizes = [256]
    rem = cap - 256
    while rem >= 616:
        sizes.append(512)
        rem -= 512
    if rem >= 208:
        sizes.extend([rem - 104, 104])
    else:
        sizes.append(rem)
    blocks, t = [], 0
    for tb in sizes:
        blocks.append((t, tb))
        t += tb
    assert t == cap, (cap, sizes)
    return blocks


def _chunks(c0, c1, step):
    out = []
    while c0 < c1:
        out.append((c0, min(c0 + step, c1)))
        c0 = out[-1][1]
    return out


# 2048 fp16 cols = 4 KiB per partition row — the DGE's best packet size.
DMA_CHUNK = 2048


def build_program(cap):
    nc = bass.Bass("TRN2", target_bir_lowering=False, debug=False, num_devices=N_CORES)
    # xP: host-packed [128, KC*cap]; token block (t0,tb) occupies columns
    # [KC*t0, KC*(t0+tb)) laid out [p, (kc t)]. yP likewise [p, (dt t)].
    # w1/w2 host-prepacked [128, 8192]: col (m*1024 + kc*128 + j) holds
    # W[kc*128+p, m*128+j] — identical layout to the SBUF tile, so DMAs are
    # plain 2D copies in any chunking.
    xP = nc.declare_dram_parameter("xP", [128, KC * cap], F16, isOutput=False)
    w1 = nc.declare_dram_parameter("w1", [128, FT * D_MODEL], F16, isOutput=False)
    w2 = nc.declare_dram_parameter("w2", [128, DT * D_FF], F16, isOutput=False)
    yP = nc.declare_dram_parameter("yP", [128, DT * cap], F16, isOutput=True)

    blocks = _token_blocks(cap)
    nb = len(blocks)

    with tile.TileContext(nc) as tc:
        with (
            tc.tile_pool(name="wpool", bufs=1) as wpool,
            tc.tile_pool(name="xpool", bufs=nb) as xpool,
            tc.tile_pool(name="mpool", bufs=2) as mpool,
            tc.tile_pool(name="tpool", bufs=4) as tpool,
            tc.tile_pool(name="opool", bufs=nb) as opool,
            tc.tile_pool(name="psum", bufs=3, space="PSUM") as psum_pool,
        ):
            w1_sb = wpool.tile([128, FT * D_MODEL], F16, tag="w1")
            w2_sb = wpool.tile([128, DT * D_FF], F16, tag="w2")

            # Warm-up: the PE clock is gated to 0.65-1.2 GHz until the HAM
            # sees ~3.4 us of sustained activity, and the DGE doesn't move
            # the first bytes until ~12 us (NEFF preamble ~7 us + ~3 us
            # doorbell-to-flow latency + first chunk). Fill the whole wait
            # with dependency-free fp32 matmuls (4 cy/col, ~0.9-1.6 us each
            # below full clock) so the PE hits the first real matmul ramped
            # and exactly when its data lands. memsets go on the vector
            # engine so gpsimd's first queue entries are the w1 doorbells.
            warm_a = wpool.tile([128, 128], F32, tag="warm_a")
            warm_x = wpool.tile([128, 256], F32, tag="warm_x")
            nc.vector.memset(warm_a[:], 0.0)
            nc.vector.memset(warm_x[:], 0.0)
            wp = psum_pool.tile([128, 256], F32, tag="warm", bufs=1)
            for _ in range(10):
                nc.tensor.matmul(wp[:], warm_a[:], warm_x[:], start=True, stop=True)

            # Early-phase DMA schedule (three ~135 GB/s queues, flow starts
            # ~10.5 us): sync carries x0 then x1/x2/... (x0's first 1024-col
            # piece lands ~12.4 us = warm-up end, so the PE starts on time);
            # gpsimd carries w1 c0-c3 then w2 then the output blocks; ACT
            # carries w1 c4-c7 then only relus. Output tile ft of layer 1
            # needs w1 chunk ft, so the two w1 queues stay just ahead of the
            # PE's ~0.85 us/tile pace through block 0.
            xs = {}
            t0b, tb0 = blocks[0]
            x0_sb = xpool.tile([128, KC * tb0], F16, tag="x", name="x0")
            xs[0] = x0_sb
            for c0, c1 in _chunks(0, KC * tb0, 1024):
                nc.sync.dma_start(x0_sb[:, c0:c1], xP[:, c0:c1])
            for m in (0, 2, 4, 6):
                nc.gpsimd.dma_start(
                    w1_sb[:, m * 1024 : (m + 1) * 1024], w1[:, m * 1024 : (m + 1) * 1024]
                )
            for m in (1, 3, 5, 7):
                nc.scalar.dma_start(
                    w1_sb[:, m * 1024 : (m + 1) * 1024], w1[:, m * 1024 : (m + 1) * 1024]
                )
            # w2 isn't needed until the first layer-2 block (~33 us); split it
            # over the two weight queues behind w1 so it never competes with
            # the x stream on sync.
            nc.gpsimd.dma_start(w2_sb[:, 0:2048], w2[:, 0:2048])
            nc.gpsimd.dma_start(w2_sb[:, 2048:4096], w2[:, 2048:4096])
            nc.scalar.dma_start(w2_sb[:, 4096:6144], w2[:, 4096:6144])
            nc.scalar.dma_start(w2_sb[:, 6144:8192], w2[:, 6144:8192])

            # Software-pipelined emission: the PE stream is in-order, so
            # emit L1(b+1) before L2(b) — the PE always has layer-1 work
            # while layer-2 weights / x blocks are still streaming.
            mids = {}

            def l1(bi):
                t0, tb = blocks[bi]
                if bi in xs:
                    x_sb = xs.pop(bi)
                else:
                    x_sb = xpool.tile([128, KC * tb], F16, tag="x", name=f"x{bi}")
                    for c0, c1 in _chunks(0, KC * tb, DMA_CHUNK):
                        nc.sync.dma_start(
                            x_sb[:, c0:c1], xP[:, KC * t0 + c0 : KC * t0 + c1]
                        )
                mid_sb = mpool.tile([128, FT * tb], F16, tag="mid", name=f"mid{bi}")
                mids[bi] = mid_sb
                for ft in range(FT):
                    ps = psum_pool.tile([128, tb], F32, tag="ps", name=f"ps{bi}_{ft}")
                    for kc in range(KC):
                        nc.tensor.matmul(
                            ps[:],
                            w1_sb[
                                :,
                                ft * D_MODEL + kc * 128 : ft * D_MODEL + kc * 128 + 128,
                            ],
                            x_sb[:, kc * tb : (kc + 1) * tb],
                            start=(kc == 0),
                            stop=(kc == KC - 1),
                        )
                    tmp = tpool.tile([128, tb], F16, tag="tmp", name=f"tmp{bi}_{ft}")
                    nc.scalar.activation(
                        tmp[:], ps[:], mybir.ActivationFunctionType.Relu
                    )
                    nc.vector.tensor_mul(
                        mid_sb[:, ft * tb : (ft + 1) * tb], tmp[:], tmp[:]
                    )

            def l2(bi):
                t0, tb = blocks[bi]
                mid_sb = mids.pop(bi)
                o_sb = opool.tile([128, DT * tb], F16, tag="o", name=f"o{bi}")
                for dt_ in range(DT):
                    ps2 = psum_pool.tile(
                        [128, tb], F32, tag="ps2", name=f"ps2{bi}_{dt_}"
                    )
                    for fc in range(FT):
                        nc.tensor.matmul(
                            ps2[:],
                            w2_sb[
                                :, dt_ * D_FF + fc * 128 : dt_ * D_FF + fc * 128 + 128
                            ],
                            mid_sb[:, fc * tb : (fc + 1) * tb],
                            start=(fc == 0),
                            stop=(fc == FT - 1),
                        )
                    nc.vector.tensor_copy(o_sb[:, dt_ * tb : (dt_ + 1) * tb], ps2[:])
                    if bi == nb - 1 and dt_ == DT // 2 - 1:
                        # tail block: ship the first half mid-block so only
                        # half the (tiny) output DMA trails the last matmul
                        nc.sync.dma_start(
                            yP[:, DT * t0 : DT * t0 + 4 * tb],
                            o_sb[:, : 4 * tb],
                        )
                if bi == nb - 1:
                    nc.sync.dma_start(
                        yP[:, DT * t0 + 4 * tb : DT * (t0 + tb)],
                        o_sb[:, 4 * tb : DT * tb],
                    )
                else:
                    for c0, c1 in _chunks(0, DT * tb, DMA_CHUNK):
                        nc.gpsimd.dma_start(
                            yP[:, DT * t0 + c0 : DT * t0 + c1], o_sb[:, c0:c1]
                        )

            LA = 1  # mid tiles live LA+1 blocks -> mpool bufs = LA+1
            for step in range(nb + LA):
                if step < nb:
                    l1(step)
                if step >= LA:
                    l2(step - LA)

    split_excess_waits(nc, limit=1)
    return nc


_PROGRAM_CACHE = {}


def _get_program(cap):
    if cap not in _PROGRAM_CACHE:
        _PROGRAM_CACHE[cap] = build_program(cap)
    return _PROGRAM_CACHE[cap]


# ---------------------------------------------------------------------------
# Host side: routing, dispatch, combine.
# ---------------------------------------------------------------------------


def _pack_blocked(aT, cap, blocks, dtype):
    """[1024, cap] feature-major -> [128, 8*cap], each token block laid out
    [p, (g t)] so the device moves one contiguous chunk per block."""
    g = aT.shape[0] // 128
    out = np.zeros((128, g * cap), dtype)
    for t0, tb in blocks:
        out[:, g * t0 : g * (t0 + tb)] = (
            aT[:, t0 : t0 + tb]
            .reshape(g, 128, tb)
            .transpose(1, 0, 2)
            .reshape(128, g * tb)
        )
    return out


def _unpack_blocked(aP, cap, blocks):
    g = aP.shape[1] // cap
    out = np.empty((g * 128, cap), np.float32)
    for t0, tb in blocks:
        blk = aP[:, g * t0 : g * (t0 + tb)].astype(np.float32).reshape(128, g, tb)
        out[:, t0 : t0 + tb] = blk.transpose(1, 0, 2).reshape(g * 128, tb)
    return out


def _prep_weight(w):
    """[K, M] -> [128, (mt kc j)] fp16: col mt*1024 + kc*128 + j holds
    W[kc*128 + p, mt*128 + j]."""
    k, m = w.shape
    return np.ascontiguousarray(
        w.reshape(k // 128, 128, m // 128, 128)
        .transpose(1, 2, 0, 3)
        .reshape(128, k * m // 128),
        dtype=np.float16,
    )


def kernel(x, Wr, W1, W2, _trace=False):
    x = np.asarray(x)
    Wr = np.asarray(Wr)
    W1 = np.asarray(W1)
    W2 = np.asarray(W2)
    B, T, C = x.shape
    N = B * T
    xf = np.ascontiguousarray(x.reshape(N, C), dtype=np.float32)

    # Router in float64 (matches jax f32 top_k selections; verified).
    logits = xf.astype(np.float64) @ Wr.astype(np.float64)
    logits -= logits.max(axis=-1, keepdims=True)
    p = np.exp(logits)
    p /= p.sum(axis=-1, keepdims=True)
    idx = np.argsort(-p, axis=-1, kind="stable")[:, :TOP_K]  # [N, K]
    wts = np.take_along_axis(p, idx, axis=-1)  # [N, K]

    # Dispatch list sorted by expert.
    flat_e = idx.ravel()
    order = np.argsort(flat_e, kind="stable")
    tok_of_pair = np.repeat(np.arange(N), TOP_K)[order]
    w_of_pair = wts.ravel()[order]
    counts = np.bincount(flat_e, minlength=NUM_EXPERTS)
    starts = np.concatenate([[0], np.cumsum(counts)[:-1]])

    cap = int(max(512, -(-int(counts.max()) // 8) * 8))
    blocks = _token_blocks(cap)

    in_maps = []
    toks_per_e = []
    for e in range(NUM_EXPERTS):
        s, c = int(starts[e]), int(counts[e])
        toks = tok_of_pair[s : s + c]
        toks_per_e.append(toks)
        ws = w_of_pair[s : s + c].astype(np.float32)
        xg = xf[toks] * np.sqrt(ws)[:, None]
        xTe = np.zeros((C, cap), np.float32)
        xTe[:, :c] = xg.T
        in_maps.append(
            {
                "xP": _pack_blocked(xTe, cap, blocks, np.float16),
                "w1": _prep_weight(W1[e]),
                "w2": _prep_weight(W2[e]),
            }
        )

    nc = _get_program(cap)
    res = run_bass_kernel_spmd(nc, in_maps, core_ids=list(range(N_CORES)), trace=_trace)

    out = np.zeros((N, C), np.float32)
    for e in range(NUM_EXPERTS):
        c = int(counts[e])
        if c:
            yT = _unpack_blocked(res.results[e]["yP"], cap, blocks)
            out[toks_per_e[e]] += yT[:, :c].T
    if _trace:
        kernel._last_exec_time_ns = res.exec_time_ns
    return out.reshape(B, T, C)
